# revision 2
# baseline (speedup 1.0000x reference)
# Bass/Trainium2 kernel for nn_Bilinear_46660524703902.
#
# Math (see reference):
#   s    = sum_n x2[n, :]                        # [R] global row-sum
#   M    = einsum('olr,r->lo', U, s)             # [L, O]
#   out  = x1 @ (M + W_l) + x2 @ W_r + N * bias  # [N, O]
#
# Distribution: data-parallel over the flattened row axis across 8 cores.
# Each core computes M_c from its local partial row-sum; M is linear in s,
# so one 64KB AllReduce of M_c yields the global M on every core.
#
# Per-core dataflow (rows_per_core = 65536, 512 tiles of 128 rows):
#   Phase A: stream x2 in 8-tile chunks: SWDGE cast-DMA fp32->bf16 (natural
#            [n, r] layout), xbar DMA-transpose each 128x128 block into a
#            resident SBUF buffer x2T [r, n] (bf16, 16.8MB), DVE row-sum
#            partials (free-axis reduce over the transposed layout).
#   M:       128 PE matmuls (one per o): M[:, o] = U'[r, (o l)]-slice.T @ s,
#            copy PSUM->SBUF, DMA to internal DRAM, AllReduce, load back,
#            A = M + W_l, cast to bf16.
#   Phase B: stream x1 the same way (cast + transpose), then per 128-row
#            tile two accumulating matmuls into PSUM [n, o]:
#              psum  = x2T_tile.T @ W_r     (ready right after phase A)
#              psum += x1T_tile.T @ A       (waits on the AllReduce)
#            DVE adds the pre-tiled N*bias, store fp32 to HBM.

import numpy as np
import ml_dtypes
from contextlib import ExitStack

N_CORES = 8
FEAT = 128  # L == R == O == 128
CHUNK = 8  # 128-row tiles per chunk

_nc_cache: dict = {}


def _build(rows_per_core: int):
    """Build + compile the per-core Bass module (same program on all cores)."""
    from concourse import bacc, mybir, tile

    f32 = mybir.dt.float32
    bf16 = mybir.dt.bfloat16
    X = mybir.AxisListType.X

    P = 128
    C = CHUNK
    assert rows_per_core % (P * C) == 0
    nch = rows_per_core // (P * C)  # chunks (64 at full size)

    nc = bacc.Bacc("TRN2", target_bir_lowering=False, debug=False,
                   num_devices=N_CORES)

    x1 = nc.dram_tensor("input_left", [rows_per_core, FEAT], f32,
                        kind="ExternalInput")
    x2 = nc.dram_tensor("input_right", [rows_per_core, FEAT], f32,
                        kind="ExternalInput")
    up = nc.dram_tensor("u_prep", [FEAT, FEAT * FEAT], bf16,
                        kind="ExternalInput")  # [r, (o l)] = U[o, l, r]
    wl = nc.dram_tensor("w_l", [FEAT, FEAT], f32, kind="ExternalInput")
    wr = nc.dram_tensor("w_r", [FEAT, FEAT], bf16, kind="ExternalInput")
    biasT = nc.dram_tensor("bias_tiled", [P, C * FEAT], f32,
                           kind="ExternalInput")  # N*bias tiled C times
    out = nc.dram_tensor("out", [rows_per_core, FEAT], f32,
                         kind="ExternalOutput")

    with tile.TileContext(nc) as tc, ExitStack() as ctx:
        consts = ctx.enter_context(tc.tile_pool(name="consts", bufs=1))
        big = ctx.enter_context(tc.tile_pool(name="big", bufs=1))
        ld2 = ctx.enter_context(tc.tile_pool(name="ld2", bufs=3))
        ld1 = ctx.enter_context(tc.tile_pool(name="ld1", bufs=3))
        x1tp = ctx.enter_context(tc.tile_pool(name="x1t", bufs=3))
        outp = ctx.enter_context(tc.tile_pool(name="outp", bufs=3))
        psum = ctx.enter_context(tc.tile_pool(name="psum", bufs=3, space="PSUM"))
        mpsum = ctx.enter_context(tc.tile_pool(name="mpsum", bufs=1, space="PSUM"))
        dram = ctx.enter_context(tc.tile_pool(name="dram", bufs=1, space="DRAM"))

        # Persistent / constant tiles
        x2t_all = big.tile([P, rows_per_core], bf16)  # [r, n] resident
        up_sb = consts.tile([FEAT, FEAT * FEAT], bf16)
        wl_sb = consts.tile([FEAT, FEAT], f32)
        wr_sb = consts.tile([FEAT, FEAT], bf16)
        bias_sb = consts.tile([P, C * FEAT], f32)
        s_cols = consts.tile([P, nch], f32)
        s_f32 = consts.tile([P, 1], f32)
        s_bf = consts.tile([P, 1], bf16)
        m_sb = consts.tile([FEAT, FEAT], f32)
        mg_sb = consts.tile([FEAT, FEAT], f32)
        a_f32 = consts.tile([FEAT, FEAT], f32)
        a_bf = consts.tile([FEAT, FEAT], bf16)

        nc.sync.dma_start(up_sb[:], up[:])
        nc.sync.dma_start(wl_sb[:], wl[:])
        nc.sync.dma_start(wr_sb[:], wr[:])
        nc.sync.dma_start(bias_sb[:], biasT[:])

        x2v = x2.ap().rearrange("(h c p) r -> h p c r", p=P, c=C)
        x1v = x1.ap().rearrange("(h c p) r -> h p c r", p=P, c=C)
        outv = out.ap().rearrange("(h c p) o -> h p c o", p=P, c=C)

        # ---------- Phase A: stream x2, transpose into residency, row-sums
        for j in range(nch):
            x2n = ld2.tile([P, C, FEAT], bf16)
            nc.gpsimd.dma_start(x2n[:], x2v[j])  # fp32 -> bf16 cast in DMA
            for c in range(C):
                t = j * C + c
                nc.scalar.dma_start_transpose(
                    x2t_all[:, t * P:(t + 1) * P], x2n[:, c, :])
            nc.vector.reduce_sum(
                s_cols[:, j:j + 1],
                x2t_all[:, j * C * P:(j + 1) * C * P], axis=X)

        # ---------- M_c = einsum(U, s_local), AllReduce -> A = M + W_l
        nc.vector.reduce_sum(s_f32[:], s_cols[:], axis=X)
        nc.vector.tensor_copy(s_bf[:], s_f32[:])
        m_ps = mpsum.tile([FEAT, FEAT], f32)
        for o in range(FEAT):
            nc.tensor.matmul(m_ps[:, o:o + 1],
                             up_sb[:, o * FEAT:(o + 1) * FEAT],
                             s_bf[:], start=True, stop=True)
        nc.vector.tensor_copy(m_sb[:], m_ps[:])
        m_loc = dram.tile([FEAT, FEAT], f32)
        m_glob = dram.tile([FEAT, FEAT], f32)
        nc.sync.dma_start(m_loc[:], m_sb[:])
        nc.gpsimd.collective_compute(
            "AllReduce", mybir.AluOpType.add,
            replica_groups=[list(range(N_CORES))],
            ins=[m_loc.opt()], outs=[m_glob.opt()])
        nc.sync.dma_start(mg_sb[:], m_glob[:])
        nc.vector.tensor_add(a_f32[:], mg_sb[:], wl_sb[:])
        nc.vector.tensor_copy(a_bf[:], a_f32[:])

        # ---------- Phase B: stream x1, matmuls, bias, store
        for j in range(nch):
            x1n = ld1.tile([P, C, FEAT], bf16)
            nc.gpsimd.dma_start(x1n[:], x1v[j])
            x1t = x1tp.tile([P, C, FEAT], bf16)  # [l, c, n]
            for c in range(C):
                nc.scalar.dma_start_transpose(x1t[:, c, :], x1n[:, c, :])
            ps = psum.tile([P, C * FEAT], f32)  # [n, (c o)]
            for c in range(C):
                t = j * C + c
                nc.tensor.matmul(ps[:, c * FEAT:(c + 1) * FEAT],
                                 x2t_all[:, t * P:(t + 1) * P],
                                 wr_sb[:], start=True, stop=False)
                nc.tensor.matmul(ps[:, c * FEAT:(c + 1) * FEAT],
                                 x1t[:, c, :],
                                 a_bf[:], start=False, stop=True)
            ob = outp.tile([P, C * FEAT], f32)
            nc.vector.tensor_add(ob[:], ps[:], bias_sb[:])
            nc.sync.dma_start(outv[j],
                              ob[:].rearrange("p (c o) -> p c o", c=C))

    nc.compile()
    return nc


def _get_nc(rows_per_core: int):
    if rows_per_core not in _nc_cache:
        _nc_cache[rows_per_core] = _build(rows_per_core)
    return _nc_cache[rows_per_core]


def make_in_maps(input_left, input_right, U, W_l, W_r, bias, n_total_rows):
    """Host-side prep: shard rows, lay out the small weights."""
    x1 = np.ascontiguousarray(np.asarray(input_left, np.float32)).reshape(-1, FEAT)
    x2 = np.ascontiguousarray(np.asarray(input_right, np.float32)).reshape(-1, FEAT)
    U = np.asarray(U, np.float32)
    rows = x1.shape[0] // N_CORES
    # up[r, o*128+l] = U[o, l, r]
    up = np.ascontiguousarray(U.transpose(2, 0, 1).reshape(FEAT, FEAT * FEAT)
                              ).astype(ml_dtypes.bfloat16)
    wl = np.ascontiguousarray(np.asarray(W_l, np.float32))
    wr = np.ascontiguousarray(np.asarray(W_r, np.float32)).astype(ml_dtypes.bfloat16)
    nb = (np.float64(n_total_rows) * np.asarray(bias, np.float64)).astype(np.float32)
    bias_tiled = np.ascontiguousarray(np.tile(nb, (128, CHUNK)))
    in_maps = []
    for c in range(N_CORES):
        in_maps.append({
            "input_left": x1[c * rows:(c + 1) * rows],
            "input_right": x2[c * rows:(c + 1) * rows],
            "u_prep": up,
            "w_l": wl,
            "w_r": wr,
            "bias_tiled": bias_tiled,
        })
    return in_maps, rows


def kernel(input_left, input_right, U, W_l, W_r, bias):
    from concourse.bass_utils import run_bass_kernel_spmd

    lead = np.asarray(input_left).shape[:-1]
    n_total = int(np.prod(lead))
    in_maps, rows = make_in_maps(input_left, input_right, U, W_l, W_r, bias,
                                 n_total)
    nc = _get_nc(rows)
    res = run_bass_kernel_spmd(nc, in_maps, core_ids=list(range(N_CORES)))
    out = np.concatenate([r["out"] for r in res.results], axis=0)
    return out.reshape(lead + (FEAT,))


# revision 3
# speedup vs baseline: 17.7276x; 17.7276x over previous
# Bass/Trainium2 kernel for nn_Bilinear_46660524703902.
#
# Math (see reference):
#   s    = sum_n x2[n, :]                        # [R] global row-sum
#   M    = einsum('olr,r->lo', U, s)             # [L, O]
#   out  = x1 @ (M + W_l) + x2 @ W_r + N * bias  # [N, O]
#
# Distribution: data-parallel over the flattened row axis across 8 cores.
# Each core computes M_c from its local partial row-sum; M is linear in s,
# so one 64KB AllReduce of M_c yields the global M on every core.
#
# Per-core dataflow (rows_per_core = 65536, 512 tiles of 128 rows):
#   Phase A: stream x2 in 8-tile chunks: SWDGE cast-DMA fp32->bf16 (natural
#            [n, r] layout), xbar DMA-transpose each 128x128 block into a
#            resident SBUF buffer x2T [r, n] (bf16, 16.8MB), DVE row-sum
#            partials (free-axis reduce over the transposed layout).
#   M:       128 PE matmuls (one per o): M[:, o] = U'[r, (o l)]-slice.T @ s,
#            copy PSUM->SBUF, DMA to internal DRAM, AllReduce, load back,
#            A = M + W_l, cast to bf16.
#   Phase B: stream x1 the same way (cast + transpose), then per 128-row
#            tile two accumulating matmuls into PSUM [n, o]:
#              psum  = x2T_tile.T @ W_r     (ready right after phase A)
#              psum += x1T_tile.T @ A       (waits on the AllReduce)
#            DVE adds the pre-tiled N*bias, store fp32 to HBM.
#
# `repeats` replicates the whole body inside one NEFF for slope timing
# (wall(R) - wall(1)) / (R - 1); repeats share buffers and serialize
# through the natural WAW/RAW dependencies.

import numpy as np
import ml_dtypes
from contextlib import ExitStack

N_CORES = 8
FEAT = 128  # L == R == O == 128
CHUNK = 8  # 128-row tiles per chunk

_nc_cache: dict = {}


def _build(rows_per_core: int, repeats: int = 1):
    """Build + compile the per-core Bass module (same program on all cores)."""
    from concourse import bacc, mybir, tile

    f32 = mybir.dt.float32
    bf16 = mybir.dt.bfloat16
    X = mybir.AxisListType.X

    P = 128
    C = CHUNK
    assert rows_per_core % (P * C) == 0
    nch = rows_per_core // (P * C)  # chunks (64 at full size)

    nc = bacc.Bacc("TRN2", target_bir_lowering=False, debug=False,
                   num_devices=N_CORES)

    x1 = nc.dram_tensor("input_left", [rows_per_core, FEAT], f32,
                        kind="ExternalInput")
    x2 = nc.dram_tensor("input_right", [rows_per_core, FEAT], f32,
                        kind="ExternalInput")
    up = nc.dram_tensor("u_prep", [FEAT, FEAT * FEAT], bf16,
                        kind="ExternalInput")  # [r, (o l)] = U[o, l, r]
    wl = nc.dram_tensor("w_l", [FEAT, FEAT], f32, kind="ExternalInput")
    wr = nc.dram_tensor("w_r", [FEAT, FEAT], bf16, kind="ExternalInput")
    biasT = nc.dram_tensor("bias_tiled", [P, C * FEAT], f32,
                           kind="ExternalInput")  # N*bias tiled C times
    out = nc.dram_tensor("out", [rows_per_core, FEAT], f32,
                         kind="ExternalOutput")

    with tile.TileContext(nc) as tc, ExitStack() as ctx:
        consts = ctx.enter_context(tc.tile_pool(name="consts", bufs=1))
        big = ctx.enter_context(tc.tile_pool(name="big", bufs=1))
        ld2 = ctx.enter_context(tc.tile_pool(name="ld2", bufs=3))
        ld1 = ctx.enter_context(tc.tile_pool(name="ld1", bufs=3))
        x1tp = ctx.enter_context(tc.tile_pool(name="x1t", bufs=3))
        outp = ctx.enter_context(tc.tile_pool(name="outp", bufs=3))
        psum = ctx.enter_context(tc.tile_pool(name="psum", bufs=3, space="PSUM"))
        mpsum = ctx.enter_context(tc.tile_pool(name="mpsum", bufs=1, space="PSUM"))
        dram = ctx.enter_context(tc.tile_pool(name="dram", bufs=1, space="DRAM"))

        # Constants, loaded once
        up_sb = consts.tile([FEAT, FEAT * FEAT], bf16)
        wl_sb = consts.tile([FEAT, FEAT], f32)
        wr_sb = consts.tile([FEAT, FEAT], bf16)
        bias_sb = consts.tile([P, C * FEAT], f32)
        nc.sync.dma_start(up_sb[:], up[:])
        nc.sync.dma_start(wl_sb[:], wl[:])
        nc.sync.dma_start(wr_sb[:], wr[:])
        nc.sync.dma_start(bias_sb[:], biasT[:])

        # Persistent working tiles (shared across repeats)
        x2t_all = big.tile([P, rows_per_core], bf16)  # [r, n] resident
        s_cols = consts.tile([P, nch], f32)
        s_f32 = consts.tile([P, 1], f32)
        s_bf = consts.tile([P, 1], bf16)
        m_sb = consts.tile([FEAT, FEAT], f32)
        mg_sb = consts.tile([FEAT, FEAT], f32)
        a_f32 = consts.tile([FEAT, FEAT], f32)
        a_bf = consts.tile([FEAT, FEAT], bf16)

        x2v = x2.ap().rearrange("(h c p) r -> h p c r", p=P, c=C)
        x1v = x1.ap().rearrange("(h c p) r -> h p c r", p=P, c=C)
        outv = out.ap().rearrange("(h c p) o -> h p c o", p=P, c=C)

        for _rep in range(repeats):
            # ------ Phase A: stream x2, transpose into residency, row-sums
            for j in range(nch):
                x2n = ld2.tile([P, C, FEAT], bf16)
                nc.gpsimd.dma_start(x2n[:], x2v[j])  # fp32 -> bf16 cast
                for c in range(C):
                    t = j * C + c
                    nc.scalar.dma_start_transpose(
                        x2t_all[:, t * P:(t + 1) * P], x2n[:, c, :])
                nc.vector.reduce_sum(
                    s_cols[:, j:j + 1],
                    x2t_all[:, j * C * P:(j + 1) * C * P], axis=X)

            # ------ M_c = einsum(U, s_local), AllReduce -> A = M + W_l
            nc.vector.reduce_sum(s_f32[:], s_cols[:], axis=X)
            nc.vector.tensor_copy(s_bf[:], s_f32[:])
            m_ps = mpsum.tile([FEAT, FEAT], f32)
            for o in range(FEAT):
                nc.tensor.matmul(m_ps[:, o:o + 1],
                                 up_sb[:, o * FEAT:(o + 1) * FEAT],
                                 s_bf[:], start=True, stop=True)
            nc.vector.tensor_copy(m_sb[:], m_ps[:])
            m_loc = dram.tile([FEAT, FEAT], f32)
            m_glob = dram.tile([FEAT, FEAT], f32)
            nc.sync.dma_start(m_loc[:], m_sb[:])
            nc.gpsimd.collective_compute(
                "AllReduce", mybir.AluOpType.add,
                replica_groups=[list(range(N_CORES))],
                ins=[m_loc.opt()], outs=[m_glob.opt()])
            nc.sync.dma_start(mg_sb[:], m_glob[:])
            nc.vector.tensor_add(a_f32[:], mg_sb[:], wl_sb[:])
            nc.vector.tensor_copy(a_bf[:], a_f32[:])

            # ------ Phase B: stream x1, matmuls, bias, store
            for j in range(nch):
                x1n = ld1.tile([P, C, FEAT], bf16)
                nc.gpsimd.dma_start(x1n[:], x1v[j])
                x1t = x1tp.tile([P, C, FEAT], bf16)  # [l, c, n]
                for c in range(C):
                    nc.scalar.dma_start_transpose(x1t[:, c, :], x1n[:, c, :])
                ps = psum.tile([P, C * FEAT], f32)  # [n, (c o)]
                for c in range(C):
                    t = j * C + c
                    nc.tensor.matmul(ps[:, c * FEAT:(c + 1) * FEAT],
                                     x2t_all[:, t * P:(t + 1) * P],
                                     wr_sb[:], start=True, stop=False)
                    nc.tensor.matmul(ps[:, c * FEAT:(c + 1) * FEAT],
                                     x1t[:, c, :],
                                     a_bf[:], start=False, stop=True)
                ob = outp.tile([P, C * FEAT], f32)
                nc.vector.tensor_add(ob[:], ps[:], bias_sb[:])
                nc.sync.dma_start(outv[j],
                                  ob[:].rearrange("p (c o) -> p c o", c=C))

    nc.compile()
    return nc


def _get_nc(rows_per_core: int, repeats: int = 1):
    key = (rows_per_core, repeats)
    if key not in _nc_cache:
        _nc_cache[key] = _build(rows_per_core, repeats)
    return _nc_cache[key]


def make_in_maps(input_left, input_right, U, W_l, W_r, bias, n_total_rows):
    """Host-side prep: shard rows, lay out the small weights."""
    x1 = np.ascontiguousarray(np.asarray(input_left, np.float32)).reshape(-1, FEAT)
    x2 = np.ascontiguousarray(np.asarray(input_right, np.float32)).reshape(-1, FEAT)
    U = np.asarray(U, np.float32)
    rows = x1.shape[0] // N_CORES
    # up[r, o*128+l] = U[o, l, r]
    up = np.ascontiguousarray(U.transpose(2, 0, 1).reshape(FEAT, FEAT * FEAT)
                              ).astype(ml_dtypes.bfloat16)
    wl = np.ascontiguousarray(np.asarray(W_l, np.float32))
    wr = np.ascontiguousarray(np.asarray(W_r, np.float32)).astype(ml_dtypes.bfloat16)
    nb = (np.float64(n_total_rows) * np.asarray(bias, np.float64)).astype(np.float32)
    bias_tiled = np.ascontiguousarray(np.tile(nb, (128, CHUNK)))
    in_maps = []
    for c in range(N_CORES):
        in_maps.append({
            "input_left": x1[c * rows:(c + 1) * rows],
            "input_right": x2[c * rows:(c + 1) * rows],
            "u_prep": up,
            "w_l": wl,
            "w_r": wr,
            "bias_tiled": bias_tiled,
        })
    return in_maps, rows


def kernel(input_left, input_right, U, W_l, W_r, bias):
    from concourse.bass_utils import run_bass_kernel_spmd

    lead = np.asarray(input_left).shape[:-1]
    n_total = int(np.prod(lead))
    in_maps, rows = make_in_maps(input_left, input_right, U, W_l, W_r, bias,
                                 n_total)
    nc = _get_nc(rows)
    res = run_bass_kernel_spmd(nc, in_maps, core_ids=list(range(N_CORES)))
    out = np.concatenate([r["out"] for r in res.results], axis=0)
    return out.reshape(lead + (FEAT,))
